# revision 1
# baseline (speedup 1.0000x reference)
"""Trainium2 Bass kernel for nn_Attention (GQA causal attention + RoPE).

Full problem: x[4,2048,2048] -> attention(16 q heads / 8 kv heads, head_dim
128, llama RoPE, causal) -> out[4,2048,2048], fp32.

Sharding across the 8 NeuronCores: tensor-parallel over heads (2 groups of
8 q / 4 kv heads; Wq/Wk/Wv column-sharded, Wo row-sharded) x data-parallel
over batch (4): core = batch*2 + head_group.  x is replicated per batch
pair; each core computes a partial [2048,2048] output which the host sums
over the 2 head-group cores per batch (the Wo all-reduce).

Per-core kernel (Bass/Tile): everything in bf16 on the matmul path with
fp32 PSUM accumulation and fp32 softmax denominators.  Q/K are projected
directly in head-transposed layout ([head_dim, tok]); RoPE pairs are
packed [even dims; odd dims] via a host-side column permutation of Wq/Wk
so rotation is 3 elementwise ops; scores are built k-major so softmax
denominators come from a ones-matmul and the attention output lands
directly in the lhsT layout the Wo projection needs (no transposes
anywhere).  Causality is handled by skipping fully-masked key blocks and
multiplying the 4 diagonal block patterns by precomputed 0/1 masks.
Projections of half the Q heads are deferred ("wave B") so the scheduler
overlaps them with the first heads' attention, keeping the tensor engine
busy through the ACT/DVE-heavy softmax stretches.
"""

import math
from contextlib import ExitStack

import numpy as np
import ml_dtypes

import concourse.bass as bass
import concourse.mybir as mybir
import concourse.tile as tile
from concourse import bacc
from concourse.bass_utils import run_bass_kernel_spmd

F32 = mybir.dt.float32
BF16 = mybir.dt.bfloat16
QW = 512   # q-chunk width
KW = 128   # k-block width

B, S, D = 4, 2048, 2048
H, KVH, HD = 16, 8, 128
NG = 2                 # head groups (tensor-parallel degree)
NQ = H // NG           # q heads per core
NKV = KVH // NG        # kv heads per core
N_CORES = 8


def _body(nc, tc, cfg, t):
    TOK, DM, NQ_, NKV_, HD_ = (cfg[k] for k in ("TOK", "DM", "NQ", "NKV", "HD"))
    DC, NTC, KB, QC, REP, ODC, SCALE = (
        cfg[k] for k in ("DC", "NTC", "KB", "QC", "REP", "ODC", "SCALE")
    )
    NQ, NKV, HD = NQ_, NKV_, HD_
    xT, wq, wk, wv, wo = t["xT"], t["wq"], t["wk"], t["wv"], t["wo"]
    cpk, spk, msk, on1, on2, out = (
        t["cpk"], t["spk"], t["msk"], t["on1"], t["on2"], t["out"]
    )
    NQA = NQ // 2  # wave-A q heads

    with ExitStack() as es:
        persist = es.enter_context(tc.tile_pool(name="persist", bufs=1))
        cpk_sb = persist.tile([HD, TOK], BF16, tag="cpk")
        spk_sb = persist.tile([HD, TOK], BF16, tag="spk")
        msk_sb = persist.tile([HD, 4 * QW], BF16, tag="msk")
        on1_sb = persist.tile([HD, 1], BF16, tag="on1")
        on2_sb = persist.tile([1, HD], F32, tag="on2")
        rotq = persist.tile([HD, NQ, TOK], BF16, tag="rotq")
        rotk = persist.tile([HD, NKV, TOK], BF16, tag="rotk")
        v_sb = persist.tile([HD, KB, NKV * HD], BF16, tag="v")

        arena = es.enter_context(tc.tile_pool(name="arena", bufs=4))
        ps12 = es.enter_context(tc.tile_pool(name="ps12", bufs=1, space="PSUM"))
        prope = es.enter_context(tc.tile_pool(name="prope", bufs=1))
        pxt = es.enter_context(tc.tile_pool(name="pxt", bufs=1))
        pe2 = es.enter_context(tc.tile_pool(name="pe2", bufs=1))
        po3p = es.enter_context(tc.tile_pool(name="po3", bufs=1))

        def big(shape, name):
            return arena.tile(shape, BF16, tag="big", name=name)

        wqA = big([HD, DC, NQA * HD], "wqA")
        wqB = big([HD, DC, NQA * HD], "wqB")
        _wqB_loaded = [False]

        def ensure_wqB():
            if _wqB_loaded[0]:
                return
            _wqB_loaded[0] = True
            for dc in range(DC):
                nc.sync.dma_start(
                    out=wqB[:, dc, :],
                    in_=wq.ap()[dc * 128:(dc + 1) * 128, NQA * HD:NQ * HD],
                )
        wk_sb = big([HD, DC, NKV * HD], "wk_sb")
        wv_sb = big([HD, DC, NKV * HD], "wv_sb")

        def load_xt(ts):
            xt = []
            for dc in range(DC):
                tt = pxt.tile([HD, QW], BF16, tag="xt", bufs=DC + 10)
                nc.sync.dma_start(
                    out=tt[:], in_=xT.ap()[dc * 128:(dc + 1) * 128, ts:ts + QW]
                )
                xt.append(tt)
            return xt

        def qk_head(w_sb, hh, dest, xt, ts, dup=None):
            """Project one q/k head for one token chunk and apply RoPE."""
            dup = dup or nc.gpsimd
            ps = ps12.tile([HD, QW], F32, tag="proj", bufs=2)
            for dc in range(DC):
                nc.tensor.matmul(
                    ps[:], w_sb[:, dc, hh * HD:(hh + 1) * HD], xt[dc][:],
                    start=(dc == 0), stop=(dc == DC - 1),
                )
            raw = prope.tile([HD, QW], BF16, tag="raw", bufs=2)
            nc.scalar.copy(raw[:], ps[:])
            qe = prope.tile([HD, QW], BF16, tag="qe", bufs=2)
            qo = prope.tile([HD, QW], BF16, tag="qo", bufs=2)
            dup.dma_start(out=qe[0:64, :], in_=raw[0:64, :])
            dup.dma_start(out=qe[64:128, :], in_=raw[0:64, :])
            dup.dma_start(out=qo[0:64, :], in_=raw[64:128, :])
            dup.dma_start(out=qo[64:128, :], in_=raw[64:128, :])
            t1 = prope.tile([HD, QW], F32, tag="rt1", bufs=2)
            t2 = prope.tile([HD, QW], F32, tag="rt2", bufs=2)
            nc.vector.tensor_mul(t1[:], qe[:], cpk_sb[:, ts:ts + QW])
            nc.vector.tensor_mul(t2[:], qo[:], spk_sb[:, ts:ts + QW])
            nc.vector.tensor_add(dest, t1[:], t2[:])

        def attention(h, attnT, hloc):
            kh = h // REP
            NSUB = QW // KW
            for j in range(QC):
                qs = j * QW
                nfull = NSUB * j  # fully-causal k-blocks for this q chunk
                po = ps12.tile([HD, QW], F32, tag="av", bufs=2)
                ea = pe2.tile([HD, QW], BF16, tag="ea", bufs=2)
                for i in range(nfull):
                    pss = ps12.tile([HD, QW], F32, tag="s", bufs=3)
                    nc.tensor.matmul(
                        pss[:],
                        rotk[:, kh, i * KW:(i + 1) * KW],
                        rotq[:, h, qs:qs + QW],
                        start=True, stop=True,
                    )
                    e = pe2.tile([HD, QW], BF16, tag="e", bufs=5)
                    nc.scalar.activation(
                        e[:], pss[:], mybir.ActivationFunctionType.Exp,
                        scale=SCALE,
                    )
                    if i == 0:
                        nc.vector.tensor_copy(ea[:], e[:])
                    else:
                        nc.vector.tensor_add(ea[:], ea[:], e[:])
                    nc.tensor.matmul(
                        po[:], v_sb[:, i, kh * HD:(kh + 1) * HD], e[:],
                        start=(i == 0), stop=False, skip_group_check=True,
                    )
                # diagonal [512x512] region: per k-row a causal suffix tile
                # [128k x (QW-128*di)q]; only its leading 128 cols need the
                # triangular mask.
                for di in range(NSUB):
                    i = nfull + di
                    w = QW - di * KW
                    sub = slice(di * KW, QW)
                    pss = ps12.tile([HD, QW], F32, tag="s", bufs=3)
                    nc.tensor.matmul(
                        pss[:, 0:w],
                        rotk[:, kh, i * KW:(i + 1) * KW],
                        rotq[:, h, qs + di * KW:qs + QW],
                        start=True, stop=True,
                    )
                    e = pe2.tile([HD, QW], BF16, tag="e", bufs=5)
                    nc.scalar.activation(
                        e[:, 0:w], pss[:, 0:w],
                        mybir.ActivationFunctionType.Exp, scale=SCALE,
                    )
                    nc.vector.tensor_mul(e[:, 0:KW], e[:, 0:KW], msk_sb[:, 0:KW])
                    if j == 0 and di == 0:
                        nc.vector.tensor_copy(ea[:, sub], e[:, 0:w])
                    else:
                        nc.vector.tensor_add(ea[:, sub], ea[:, sub], e[:, 0:w])
                    nc.tensor.matmul(
                        po[:, sub], v_sb[:, i, kh * HD:(kh + 1) * HD], e[:, 0:w],
                        start=(j == 0 and di == 0), stop=True,
                        skip_group_check=True,
                    )
                pd = ps12.tile([1, QW], F32, tag="db", bufs=1)
                nc.tensor.matmul(pd[:], on1_sb[:], ea[:], start=True, stop=True)
                rec = pe2.tile([1, QW], F32, tag="rec", bufs=2)
                nc.vector.reciprocal(rec[:], pd[:])
                pb = ps12.tile([HD, QW], F32, tag="db", bufs=1)
                nc.tensor.matmul(pb[:], on2_sb[:], rec[:], start=True, stop=True)
                bc = pe2.tile([HD, QW], F32, tag="bcs", bufs=2)
                nc.vector.tensor_copy(bc[:], pb[:])
                nc.vector.tensor_mul(attnT[:, hloc, qs:qs + QW], po[:], bc[:])

        # ---------------- wave A ----------------
        xt0 = []
        for dc in range(DC):
            nc.sync.dma_start(
                out=wv_sb[:, dc, :], in_=wv.ap()[dc * 128:(dc + 1) * 128, :]
            )
            tt = pxt.tile([HD, QW], BF16, tag="xt", bufs=DC + 10, name="xt0")
            nc.sync.dma_start(out=tt[:], in_=xT.ap()[dc * 128:(dc + 1) * 128, 0:QW])
            xt0.append(tt)
        for dc in range(DC):
            nc.sync.dma_start(
                out=wk_sb[:, dc, :], in_=wk.ap()[dc * 128:(dc + 1) * 128, :]
            )
        nc.sync.dma_start(out=cpk_sb[:], in_=cpk.ap()[:])
        nc.sync.dma_start(out=spk_sb[:], in_=spk.ap()[:])
        nc.sync.dma_start(out=msk_sb[:], in_=msk.ap()[:])
        nc.sync.dma_start(out=on1_sb[:], in_=on1.ap()[:])
        nc.sync.dma_start(out=on2_sb[:], in_=on2.ap()[:])
        for dc in range(DC):
            nc.sync.dma_start(
                out=wqA[:, dc, :],
                in_=wq.ap()[dc * 128:(dc + 1) * 128, 0:NQA * HD],
            )

        for tci in range(NTC):
            ts = tci * QW
            xt = xt0 if tci == 0 else load_xt(ts)
            # V projection (token-major tiles, all kv heads wide)
            for tb in range(QW // KW):
                gtb = tci * (QW // KW) + tb
                psv = ps12.tile([HD, NKV * HD], F32, tag="proj", bufs=2)
                for dc in range(DC):
                    nc.tensor.matmul(
                        psv[:], xt[dc][:, tb * KW:(tb + 1) * KW],
                        wv_sb[:, dc, :],
                        start=(dc == 0), stop=(dc == DC - 1),
                    )
                nc.vector.tensor_copy(v_sb[:, gtb, :], psv[:])
            for kv in range(NKV):
                qk_head(wk_sb, kv, rotk[:, kv, ts:ts + QW], xt, ts)
            # wave A: q heads 0..NQA inclusive (one head from wqB) so the
            # first attention wave covers NQA+1 heads
            for h in range(min(NQA + 1, NQ)):
                if h >= NQA:
                    ensure_wqB()
                w_, off_ = (wqA, h) if h < NQA else (wqB, h - NQA)
                qk_head(w_, off_, rotq[:, h, ts:ts + QW], xt, ts)

        # ---------------- pipelined pairs: proj(p) || attention(p-1) ----
        attnTa = big([HD, NQA, TOK], "attnTa")
        attnTb = big([HD, NQ - NQA, TOK], "attnTb")

        def attn_of(h):
            if h < NQA:
                attention(h, attnTa, h)
            else:
                attention(h, attnTb, h - NQA)

        for h in range(min(NQA + 1, NQ)):
            attn_of(h)
        for tci in range(NTC):
            ts = tci * QW
            if NQA + 1 >= NQ:
                break
            xt = load_xt(ts)
            for h in range(NQA + 1, NQ):
                ensure_wqB()
                qk_head(wqB, h - NQA, rotq[:, h, ts:ts + QW], xt, ts)

        # wo loads overlap the last attention wave
        woA = big([HD, NQA, DM], "woA")
        woB = big([HD, NQ - NQA, DM], "woB")
        for h in range(NQ):
            w_t = woA if h < NQA else woB
            nc.sync.dma_start(
                out=w_t[:, h % NQA, :], in_=wo.ap()[h * HD:(h + 1) * HD, :]
            )
        for h in range(min(NQA + 1, NQ), NQ):
            attn_of(h)

        # ---------------- output projection ----------------
        n_copy = 0
        for tb in range(KB):
            for oc in range(ODC):
                po3 = ps12.tile([HD, QW], F32, tag="s", bufs=3)
                for h in range(NQ):
                    a_t = attnTa if h < NQA else attnTb
                    w_t = woA if h < NQA else woB
                    nc.tensor.matmul(
                        po3[:],
                        a_t[:, h % NQA, tb * KW:(tb + 1) * KW],
                        w_t[:, h % NQA, oc * QW:(oc + 1) * QW],
                        start=(h == 0), stop=(h == NQ - 1),
                    )
                ot = po3p.tile([HD, QW], F32, tag="ot", bufs=5)
                if n_copy % 2 == 0:
                    nc.vector.tensor_copy(ot[:], po3[:])
                else:
                    nc.scalar.copy(ot[:], po3[:])
                n_copy += 1
                nc.sync.dma_start(
                    out=out.ap()[tb * KW:(tb + 1) * KW, oc * QW:(oc + 1) * QW],
                    in_=ot[:],
                )


def build(TOK=S, DM=D, NQ_=NQ, NKV_=NKV, reps=1):
    cfg = dict(
        TOK=TOK, DM=DM, NQ=NQ_, NKV=NKV_, HD=HD,
        DC=DM // 128, NTC=TOK // QW, KB=TOK // KW, QC=TOK // QW,
        REP=NQ_ // NKV_, ODC=DM // QW, SCALE=1.0 / math.sqrt(HD),
    )
    nc = bacc.Bacc("TRN2", target_bir_lowering=False, debug=False)
    t = {}
    t["xT"] = nc.dram_tensor("xT", [DM, TOK], BF16, kind="ExternalInput")
    t["wq"] = nc.dram_tensor("wq", [DM, NQ_ * HD], BF16, kind="ExternalInput")
    t["wk"] = nc.dram_tensor("wk", [DM, NKV_ * HD], BF16, kind="ExternalInput")
    t["wv"] = nc.dram_tensor("wv", [DM, NKV_ * HD], BF16, kind="ExternalInput")
    t["wo"] = nc.dram_tensor("wo", [NQ_ * HD, DM], BF16, kind="ExternalInput")
    t["cpk"] = nc.dram_tensor("cpk", [HD, TOK], BF16, kind="ExternalInput")
    t["spk"] = nc.dram_tensor("spk", [HD, TOK], BF16, kind="ExternalInput")
    t["msk"] = nc.dram_tensor("msk", [HD, 4 * QW], BF16, kind="ExternalInput")
    t["on1"] = nc.dram_tensor("on1", [HD, 1], BF16, kind="ExternalInput")
    t["on2"] = nc.dram_tensor("on2", [1, HD], F32, kind="ExternalInput")
    t["out"] = nc.dram_tensor("out", [TOK, DM], F32, kind="ExternalOutput")
    with tile.TileContext(nc) as tc:
        for _ in range(reps):
            _body(nc, tc, cfg, t)
    nc.compile()
    return nc


# ---------------- host-side sharding ----------------

def _rope_perm():
    return np.concatenate([np.arange(0, 128, 2), np.arange(1, 128, 2)])


def _make_masks():
    k = np.arange(128)[:, None]
    q = np.arange(512)[None, :]
    out = np.zeros((128, 4 * 512), np.float32)
    for m in range(4):
        out[:, m * 512:(m + 1) * 512] = (128 * m + k <= q).astype(np.float32)
    return out.astype(ml_dtypes.bfloat16)


def shard_inputs(x, freqs_cos, freqs_sin, Wq, Wk, Wv, Wo):
    """Per-core in_maps; core = batch*NG + head_group."""
    bf = ml_dtypes.bfloat16
    perm = _rope_perm()
    masks = _make_masks()
    cpk = np.concatenate([freqs_cos.T, freqs_sin.T], axis=0).astype(bf)
    spk = np.concatenate([-freqs_sin.T, freqs_cos.T], axis=0).astype(bf)
    on1 = np.ones((128, 1), ml_dtypes.bfloat16)
    on2 = np.ones((1, 128), np.float32)

    in_maps = []
    for b in range(B):
        xt = np.ascontiguousarray(np.asarray(x)[b].T).astype(bf)
        for g in range(NG):
            qh = slice(g * NQ * 128, (g + 1) * NQ * 128)
            kvh = slice(g * NKV * 128, (g + 1) * NKV * 128)
            wq_g = Wq[:, qh].reshape(-1, NQ, 128)[:, :, perm].reshape(-1, NQ * 128)
            wk_g = Wk[:, kvh].reshape(-1, NKV, 128)[:, :, perm].reshape(-1, NKV * 128)
            in_maps.append(dict(
                xT=xt,
                wq=np.ascontiguousarray(wq_g).astype(bf),
                wk=np.ascontiguousarray(wk_g).astype(bf),
                wv=np.ascontiguousarray(Wv[:, kvh]).astype(bf),
                wo=np.ascontiguousarray(Wo[qh, :]).astype(bf),
                cpk=cpk, spk=spk, msk=masks, on1=on1, on2=on2,
            ))
    return in_maps


_NC_CACHE = {}


def kernel(x, freqs_cos, freqs_sin, Wq, Wk, Wv, Wo):
    """Full-problem entry point: full inputs in, full [B,S,D] fp32 out."""
    if "nc" not in _NC_CACHE:
        _NC_CACHE["nc"] = build()
    nc = _NC_CACHE["nc"]
    in_maps = shard_inputs(
        np.asarray(x), np.asarray(freqs_cos), np.asarray(freqs_sin),
        np.asarray(Wq), np.asarray(Wk), np.asarray(Wv), np.asarray(Wo),
    )
    res = run_bass_kernel_spmd(nc, in_maps, core_ids=list(range(N_CORES)))
    out = np.zeros((B, S, D), np.float32)
    for b in range(B):
        out[b] = res.results[b * NG]["out"] + res.results[b * NG + 1]["out"]
    return out

